# revision 16
# baseline (speedup 1.0000x reference)
"""Trainium2 Bass kernel for nn_DenseProduct (num_factors=2).

Computes, for input x of shape (128, 16, 64, 32) f32:
    out[s, d, b, i*32+j] = x[2s, d, b, i] + x[2s+1, d, b, j]
with output shape (64, 16, 64, 1024) f32.

Sharding: scope axis (dim 0) across 8 NeuronCores — core c handles output
scopes [8c, 8c+8).

The kernel is output-write bound, so the whole game is shrinking and
overlapping HBM traffic within the rel-err budget (2e-2):

1. bf16 output (rel err ~4e-3) halves the f32 write traffic.
2. DVE 2x_1p perf mode requires every operand's innermost AP dim to be
   stride +-1 with a 2-byte dtype. A plain broadcast outer-sum always
   leaves one operand with innermost stride 0; iterating the 32x32 tile
   along wrap-around diagonals with a doubled c (c2 = [c|c]),
       out2[p, (bl, dd, t)] = a[p, (bl, t)] + c2[p, (bl, dd + t)]
   makes every operand innermost stride-1, so one tensor_tensor per scope
   runs at 2 elem/cycle/lane. out2 holds out_std[i=t, j=(dd+t)%32]; the
   host undoes the permutation with one gather on the last axis. The
   input arrives host-packed as rows of [a(32) | c(32) | c(32)] so the
   doubled-c window needs no on-device prep.
3. The second half (bl 4-7) of scopes 1-6 ships as uint8 under a global
   affine code u = round((v - lo)/step + 0.5), step = (hi-lo)/254, with
   lo/hi exact bounds of the bf16 outer sums (outer-sum extrema = row
   max + row max, computed on the host from x in O(input)). The
   otherwise-idle Scalar/ACT engine does the affine+convert (Copy
   activation with scale/bias immediates) off the DVE critical path,
   halving those bytes again (abs err ~1 step ~ 0.07, rel ~1e-2).
   Half-scope granularity keeps the DMA stream fed: after every u8 TT
   piece the next bf16 piece is at most 2.4us away, and the ACT convert
   chain finishes together with the bf16 tail.
"""

import numpy as np
import ml_dtypes

_S_IN = 128        # total input scopes
_NF = 2            # num_factors (hardcoded)
_S_OUT = _S_IN // _NF
_D = 16
_B = 64
_N = 32
_N_CORES = 8
_S_LOC = _S_OUT // _N_CORES    # 8 output scopes per core
_P = 128
_BH = 8
_BL = 8
_K = 3 * _N                    # input row: a | c | c
_FREE_IN = _BL * _K            # 768
_FREE_OUT = _BL * _N * _N      # 8192 per partition per scope
_HALF = _FREE_OUT // 2

_U8_SCOPES = (0, 1, 2, 3, 4, 5, 6)   # these scopes' bl 4-7 half ships as uint8

_CACHE = {}
LAST_RESULTS = None  # BassKernelResults of the most recent run (for profiling)


def _diag_unperm():
    """index vector g: out_std[..., k] = out2[..., g[k]]."""
    k = np.arange(_N * _N)
    i = k // _N
    j = k % _N
    dd = (j - i) % _N
    return (dd * _N + i).astype(np.int64)


def _build_bass(scale, bias):
    import concourse.bacc as bacc
    import concourse.mybir as mybir
    from concourse.ap import AP
    from concourse.tile import TileContext

    nc = bacc.Bacc("TRN2", target_bir_lowering=False, debug=False,
                   num_devices=_N_CORES)
    x = nc.dram_tensor("x", [_S_LOC, _D, _B, _K], mybir.dt.bfloat16,
                       kind="ExternalInput").ap()
    ob = nc.dram_tensor("ob", [_S_LOC, _P, _FREE_OUT],
                        mybir.dt.bfloat16, kind="ExternalOutput").ap()
    ou = nc.dram_tensor("ou", [len(_U8_SCOPES), _P, _HALF],
                        mybir.dt.uint8, kind="ExternalOutput").ap()
    u8_idx = {s: k for k, s in enumerate(_U8_SCOPES)}

    with TileContext(nc) as tc:
        with tc.tile_pool(name="head", bufs=1) as head_pool, \
             tc.tile_pool(name="in0", bufs=1) as in0_pool, \
             tc.tile_pool(name="in12", bufs=1) as in12_pool, \
             tc.tile_pool(name="inrest", bufs=1) as rest_pool, \
             tc.tile_pool(name="outp", bufs=4) as out_pool, \
             tc.tile_pool(name="u8p", bufs=3) as u8_pool:
            # x[s, d, 8*bh+bl, k] -> partition (d, bh), free (s, (bl, k))
            xr = x.rearrange("s d (bh bl) k -> (d bh) s (bl k)", bh=_BH)
            # Input issue plan: each DMA_DIRECT2D occupies its issuing engine
            # ~0.6us, so spread issues over both engines. The tiny head strip
            # (scope 0, bl=0..1: 48 KB) lands first so the first TT piece
            # (and with it the output stream) starts as early as possible.
            head = head_pool.tile([_P, 2 * _K], mybir.dt.bfloat16)
            nc.sync.dma_start(out=head[:, :], in_=xr[:, 0][:, 0:2 * _K])
            t0 = in0_pool.tile([_P, _FREE_IN], mybir.dt.bfloat16)
            nc.scalar.dma_start(out=t0[:, :], in_=xr[:, 0])
            t12 = in12_pool.tile([_P, 2 * _FREE_IN], mybir.dt.bfloat16)
            nc.sync.dma_start(out=t12[:, :].rearrange("p (s f) -> p s f", s=2),
                              in_=xr[:, 1:3])
            trest = rest_pool.tile([_P, 5 * _FREE_IN], mybir.dt.bfloat16)
            nc.scalar.dma_start(
                out=trest[:, :].rearrange("p (s f) -> p s f", s=5),
                in_=xr[:, 3:_S_LOC])

            def in_src(s):
                if s in (1, 2):
                    return t12[:, (s - 1) * _FREE_IN:s * _FREE_IN]
                return trest[:, (s - 3) * _FREE_IN:(s - 2) * _FREE_IN]

            # bf16 piece ring plan: sync by default (the scalar/ACT engine is
            # busy converting u8 halves mid-kernel, so bf16 issues queued
            # there would stall); ACT's ring gets the early scope-0 pieces
            # (it is idle until the first u8 TT lands) and the kernel-tail
            # pieces (idle again after the last convert). Mid-stream sync
            # completion boundaries hide under the concurrent u8 stream.
            for s in range(_S_LOC):
                if s == 0:
                    pieces = [(0, 1), (1, 1), (2, 2), (4, 4)]
                    rings = [nc.sync, nc.scalar, nc.sync, nc.scalar]
                elif s == _S_LOC - 1:
                    pieces = [(0, 3), (3, 3), (6, 2)]
                    rings = [nc.sync, nc.scalar, nc.scalar]
                else:
                    pieces = [(0, 4), (4, 4)]
                    rings = [nc.sync, nc.sync]
                ot = out_pool.tile([_P, _FREE_OUT], mybir.dt.bfloat16)
                for (bl0, w), ring in zip(pieces, rings):
                    # out2[p, bl, dd, t] = a[p, bl, t] + c2[p, bl, dd + t]
                    if s == 0 and bl0 < 2:
                        src = head[:, :]
                    else:
                        src = t0[:, :] if s == 0 else in_src(s)
                    pa = list(src.ap[0])
                    a = AP(src.tensor, src.offset + bl0 * _K,
                           [pa, [_K, w], [0, _N], [1, _N]])
                    c2 = AP(src.tensor, src.offset + bl0 * _K + _N,
                            [pa, [_K, w], [1, _N], [1, _N]])
                    f0 = bl0 * _N * _N
                    sz = w * _N * _N
                    o4 = ot[:, f0:f0 + sz].rearrange(
                        "p (bl dd t) -> p bl dd t", bl=w, dd=_N)
                    nc.vector.tensor_add(o4, a, c2)
                    u8_half = s in u8_idx and bl0 == 4
                    if not u8_half:
                        ring.dma_start(out=ob[s][:, f0:f0 + sz],
                                       in_=ot[:, f0:f0 + sz])
                if s in u8_idx:
                    # ACT affine-converts the bl 4-7 half to uint8 and issues
                    # its DMA on the scalar ring.
                    ut = u8_pool.tile([_P, _HALF], mybir.dt.uint8)
                    nc.scalar.activation(
                        ut[:, :], ot[:, _HALF:],
                        mybir.ActivationFunctionType.Copy,
                        bias=float(bias), scale=float(scale))
                    nc.scalar.dma_start(out=ou[u8_idx[s]], in_=ut[:, :])
    nc.compile()
    return nc


def kernel(x, num_factors):
    global LAST_RESULTS
    from concourse.bass_utils import run_bass_kernel_spmd

    x = np.asarray(x)
    assert x.shape == (_S_IN, _D, _B, _N), x.shape
    assert int(num_factors) == _NF, num_factors

    xb = x.astype(ml_dtypes.bfloat16)
    a = xb[0::2]   # [64, 16, 64, 32] factor-0 rows per output scope
    c = xb[1::2]   # factor-1 rows
    inp = np.concatenate([a, c, c], axis=-1)   # [64, 16, 64, 96]

    # exact bounds of the bf16 outer sums: rowwise max(a)+max(c)
    af = a.astype(np.float32)
    cf = c.astype(np.float32)
    hi = float((af.max(-1) + cf.max(-1)).max())
    lo = float((af.min(-1) + cf.min(-1)).min())
    step = max((hi - lo) / 254.0, 1e-30)
    scale = 1.0 / step
    bias = -lo * scale + 0.5

    key = ("nc", round(scale, 9), round(bias, 9))
    if _CACHE.get("key") != key:
        _CACHE["nc"] = _build_bass(scale, bias)
        _CACHE["key"] = key
        _CACHE["g"] = _diag_unperm()
    nc = _CACHE["nc"]

    in_maps = [
        {"x": np.ascontiguousarray(inp[cc * _S_LOC:(cc + 1) * _S_LOC])}
        for cc in range(_N_CORES)
    ]
    res = run_bass_kernel_spmd(nc, in_maps, core_ids=list(range(_N_CORES)))
    LAST_RESULTS = res

    v = np.empty((_N_CORES, _S_LOC, _P, _FREE_OUT), np.float32)
    for cc in range(_N_CORES):
        v[cc] = np.asarray(res.results[cc]["ob"])
        ru = np.asarray(res.results[cc]["ou"]).astype(np.float32)
        v[cc, list(_U8_SCOPES), :, _HALF:] = ru * step + lo
    # [64, P=(d, bh), (bl, dd, t)] -> [64, d, bh, bl, 1024(dd,t)]
    raw = v.reshape(_S_OUT, _D, _BH, _BL, _N * _N)
    out = raw[..., _CACHE["g"]]
    return np.ascontiguousarray(out).reshape(_S_OUT, _D, _B, _N ** _NF)


# revision 17
# speedup vs baseline: 1.0144x; 1.0144x over previous
"""Trainium2 Bass kernel for nn_DenseProduct (num_factors=2).

Computes, for input x of shape (128, 16, 64, 32) f32:
    out[s, d, b, i*32+j] = x[2s, d, b, i] + x[2s+1, d, b, j]
with output shape (64, 16, 64, 1024) f32.

Sharding: scope axis (dim 0) across 8 NeuronCores — core c handles output
scopes [8c, 8c+8).

The kernel is output-write bound, so the whole game is shrinking and
overlapping HBM traffic within the rel-err budget (2e-2):

1. bf16 output (rel err ~4e-3) halves the f32 write traffic.
2. DVE 2x_1p perf mode requires every operand's innermost AP dim to be
   stride +-1 with a 2-byte dtype. A plain broadcast outer-sum always
   leaves one operand with innermost stride 0; iterating the 32x32 tile
   along wrap-around diagonals with a doubled c (c2 = [c|c]),
       out2[p, (bl, dd, t)] = a[p, (bl, t)] + c2[p, (bl, dd + t)]
   makes every operand innermost stride-1, so one tensor_tensor per scope
   runs at 2 elem/cycle/lane. out2 holds out_std[i=t, j=(dd+t)%32]; the
   host undoes the permutation with one gather on the last axis. The
   input arrives host-packed as rows of [a(32) | c(32) | c(32)] so the
   doubled-c window needs no on-device prep.
3. The second half (bl 4-7) of scopes 1-6 ships as uint8 under a global
   affine code u = round((v - lo)/step + 0.5), step = (hi-lo)/254, with
   lo/hi exact bounds of the bf16 outer sums (outer-sum extrema = row
   max + row max, computed on the host from x in O(input)). The
   otherwise-idle Scalar/ACT engine does the affine+convert (Copy
   activation with scale/bias immediates) off the DVE critical path,
   halving those bytes again (abs err ~1 step ~ 0.07, rel ~1e-2).
   Half-scope granularity keeps the DMA stream fed: after every u8 TT
   piece the next bf16 piece is at most 2.4us away, and the ACT convert
   chain finishes together with the bf16 tail.
"""

import numpy as np
import ml_dtypes

_S_IN = 128        # total input scopes
_NF = 2            # num_factors (hardcoded)
_S_OUT = _S_IN // _NF
_D = 16
_B = 64
_N = 32
_N_CORES = 8
_S_LOC = _S_OUT // _N_CORES    # 8 output scopes per core
_P = 128
_BH = 8
_BL = 8
_K = 3 * _N                    # input row: a | c | c
_FREE_IN = _BL * _K            # 768
_FREE_OUT = _BL * _N * _N      # 8192 per partition per scope
_HALF = _FREE_OUT // 2

_U8_SCOPES = (0, 1, 2, 3, 4, 5, 6)   # these scopes' bl 4-7 half ships as uint8

_CACHE = {}
LAST_RESULTS = None  # BassKernelResults of the most recent run (for profiling)


def _diag_unperm():
    """index vector g: out_std[..., k] = out2[..., g[k]]."""
    k = np.arange(_N * _N)
    i = k // _N
    j = k % _N
    dd = (j - i) % _N
    return (dd * _N + i).astype(np.int64)


def _build_bass(scale, bias):
    import concourse.bacc as bacc
    import concourse.mybir as mybir
    from concourse.ap import AP
    from concourse.tile import TileContext

    nc = bacc.Bacc("TRN2", target_bir_lowering=False, debug=False,
                   num_devices=_N_CORES)
    x = nc.dram_tensor("x", [_S_LOC, _D, _B, _K], mybir.dt.bfloat16,
                       kind="ExternalInput").ap()
    ob = nc.dram_tensor("ob", [_S_LOC, _P, _FREE_OUT],
                        mybir.dt.bfloat16, kind="ExternalOutput").ap()
    ou = nc.dram_tensor("ou", [len(_U8_SCOPES), _P, _HALF],
                        mybir.dt.uint8, kind="ExternalOutput").ap()
    u8_idx = {s: k for k, s in enumerate(_U8_SCOPES)}

    with TileContext(nc) as tc:
        with tc.tile_pool(name="head", bufs=1) as head_pool, \
             tc.tile_pool(name="in0", bufs=1) as in0_pool, \
             tc.tile_pool(name="in12", bufs=1) as in12_pool, \
             tc.tile_pool(name="inrest", bufs=1) as rest_pool, \
             tc.tile_pool(name="outp", bufs=4) as out_pool, \
             tc.tile_pool(name="u8p", bufs=3) as u8_pool:
            # x[s, d, 8*bh+bl, k] -> partition (d, bh), free (s, (bl, k))
            xr = x.rearrange("s d (bh bl) k -> (d bh) s (bl k)", bh=_BH)
            # Input issue plan: each DMA_DIRECT2D occupies its issuing engine
            # ~0.6us, so spread issues over both engines. The tiny head strip
            # (scope 0, bl=0..1: 48 KB) lands first so the first TT piece
            # (and with it the output stream) starts as early as possible.
            head = head_pool.tile([_P, 2 * _K], mybir.dt.bfloat16)
            nc.sync.dma_start(out=head[:, :], in_=xr[:, 0][:, 0:2 * _K])
            t0 = in0_pool.tile([_P, _FREE_IN], mybir.dt.bfloat16)
            nc.scalar.dma_start(out=t0[:, :], in_=xr[:, 0])
            t12 = in12_pool.tile([_P, 2 * _FREE_IN], mybir.dt.bfloat16)
            nc.sync.dma_start(out=t12[:, :].rearrange("p (s f) -> p s f", s=2),
                              in_=xr[:, 1:3])
            trest = rest_pool.tile([_P, 5 * _FREE_IN], mybir.dt.bfloat16)
            nc.scalar.dma_start(
                out=trest[:, :].rearrange("p (s f) -> p s f", s=5),
                in_=xr[:, 3:_S_LOC])

            def in_src(s):
                if s in (1, 2):
                    return t12[:, (s - 1) * _FREE_IN:s * _FREE_IN]
                return trest[:, (s - 3) * _FREE_IN:(s - 2) * _FREE_IN]

            # bf16 piece ring plan: sync by default (the scalar/ACT engine is
            # busy converting u8 halves mid-kernel, so bf16 issues queued
            # there would stall); ACT's ring gets the early scope-0 pieces
            # (it is idle until the first u8 TT lands) and the kernel-tail
            # pieces (idle again after the last convert). Mid-stream sync
            # completion boundaries hide under the concurrent u8 stream.
            for s in range(_S_LOC):
                if s == 0:
                    pieces = [(0, 1), (1, 1), (2, 2), (4, 4)]
                    rings = [nc.sync, nc.scalar, nc.sync, nc.scalar]
                elif s == _S_LOC - 1:
                    pieces = [(0, 3), (3, 3), (6, 2)]
                    rings = [nc.sync, nc.scalar, nc.scalar]
                else:
                    pieces = [(0, 4), (4, 4)]
                    rings = [nc.sync, nc.sync]
                ot = out_pool.tile([_P, _FREE_OUT], mybir.dt.bfloat16)
                for (bl0, w), ring in zip(pieces, rings):
                    # out2[p, bl, dd, t] = a[p, bl, t] + c2[p, bl, dd + t]
                    if s == 0 and bl0 < 2:
                        src = head[:, :]
                    else:
                        src = t0[:, :] if s == 0 else in_src(s)
                    pa = list(src.ap[0])
                    a = AP(src.tensor, src.offset + bl0 * _K,
                           [pa, [_K, w], [0, _N], [1, _N]])
                    c2 = AP(src.tensor, src.offset + bl0 * _K + _N,
                            [pa, [_K, w], [1, _N], [1, _N]])
                    f0 = bl0 * _N * _N
                    sz = w * _N * _N
                    o4 = ot[:, f0:f0 + sz].rearrange(
                        "p (bl dd t) -> p bl dd t", bl=w, dd=_N)
                    nc.vector.tensor_add(o4, a, c2)
                    u8_half = s in u8_idx and bl0 == 4
                    if not u8_half:
                        ring.dma_start(out=ob[s][:, f0:f0 + sz],
                                       in_=ot[:, f0:f0 + sz])
                if s in u8_idx:
                    # ACT affine-converts the bl 4-7 half to uint8 and issues
                    # its DMA on the scalar ring.
                    ut = u8_pool.tile([_P, _HALF], mybir.dt.uint8)
                    nc.scalar.activation(
                        ut[:, :], ot[:, _HALF:],
                        mybir.ActivationFunctionType.Copy,
                        bias=float(bias), scale=float(scale))
                    nc.scalar.dma_start(out=ou[u8_idx[s]], in_=ut[:, :])
    nc.compile()
    return nc


def kernel(x, num_factors):
    global LAST_RESULTS
    from concourse.bass_utils import run_bass_kernel_spmd

    x = np.asarray(x)
    assert x.shape == (_S_IN, _D, _B, _N), x.shape
    assert int(num_factors) == _NF, num_factors

    xb = x.astype(ml_dtypes.bfloat16)
    a = xb[0::2]   # [64, 16, 64, 32] factor-0 rows per output scope
    c = xb[1::2]   # factor-1 rows
    inp = np.concatenate([a, c, c], axis=-1)   # [64, 16, 64, 96]

    # exact bounds of the bf16 outer sums: rowwise max(a)+max(c)
    af = a.astype(np.float32)
    cf = c.astype(np.float32)
    hi = float((af.max(-1) + cf.max(-1)).max())
    lo = float((af.min(-1) + cf.min(-1)).min())
    step = max((hi - lo) / 254.0, 1e-30)
    scale = 1.0 / step
    bias = -lo * scale + 0.5

    key = ("nc", round(scale, 9), round(bias, 9))
    if _CACHE.get("key") != key:
        _CACHE["nc"] = _build_bass(scale, bias)
        _CACHE["key"] = key
        _CACHE["g"] = _diag_unperm()
    nc = _CACHE["nc"]

    in_maps = [
        {"x": np.ascontiguousarray(inp[cc * _S_LOC:(cc + 1) * _S_LOC])}
        for cc in range(_N_CORES)
    ]
    res = run_bass_kernel_spmd(nc, in_maps, core_ids=list(range(_N_CORES)))
    LAST_RESULTS = res

    v = np.empty((_N_CORES, _S_LOC, _P, _FREE_OUT), np.float32)
    for cc in range(_N_CORES):
        v[cc] = np.asarray(res.results[cc]["ob"])
        ru = np.asarray(res.results[cc]["ou"]).astype(np.float32)
        v[cc, list(_U8_SCOPES), :, _HALF:] = (ru - 0.5) * step + lo
    # [64, P=(d, bh), (bl, dd, t)] -> [64, d, bh, bl, 1024(dd,t)]
    raw = v.reshape(_S_OUT, _D, _BH, _BL, _N * _N)
    out = raw[..., _CACHE["g"]]
    return np.ascontiguousarray(out).reshape(_S_OUT, _D, _B, _N ** _NF)
